# revision 34
# baseline (speedup 1.0000x reference)
"""Multi-head self-attention (S=2048, B=2, D=1024, H=16) on 8 TRN2 NeuronCores.

Sharding: core c handles batch b = c//4 and head-quad g = c%4 (4 heads of 64).
Megatron-style: in_proj column-sliced, out_proj row-sliced; host sums the 8
partial outputs and adds out_proj bias.

Per-core dataflow (matmul inputs bf16, accumulation fp32):
  - host supplies x^T (D-major) activations and pre-transposed weight slices
  - qpT/kpT computed head-major (m on partitions, seq on free)
  - vp computed seq-major with an interleaved ones column per head (65-wide
    blocks) so the PV matmul also produces softmax row-sums on partition 64
  - scores^T per (head-pair, 512-query-chunk, key-tile) in a packed psum tile
    (128, 2, 512); exp on ACT reads the pair in one op
  - normalization: K=1 matmul broadcasts the row-sums, DVE divides
  - out-projection on device from attn^T; bias + cross-core reduction on host
"""

import math
from contextlib import ExitStack, nullcontext as _null_ctx

import numpy as np

S = 2048
B = 2
D = 1024
H = 16
DK = 64
HC = 4          # heads per core
M = HC * DK     # 256 head-dim columns per core
N_CORES = 8
KT = S // 128   # 16 key tiles
QQ = 4          # 512-wide query chunks

MM_DT = "bfloat16"   # dtype of matmul inputs ("bfloat16" or "float32r")

_compiled = None


def _build_program():
    import concourse.tile as tile
    from concourse import mybir, bacc

    f32 = mybir.dt.float32
    f32r = mybir.dt.float32r
    mdt = getattr(mybir.dt, MM_DT)
    EXP = mybir.ActivationFunctionType.Exp

    nc = bacc.Bacc("TRN2", target_bir_lowering=False, debug=False)

    xqT = nc.dram_tensor("xqT", [D, S], mdt, kind="ExternalInput").ap()
    xkT = nc.dram_tensor("xkT", [D, S], mdt, kind="ExternalInput").ap()
    xvT = nc.dram_tensor("xvT", [D, S], mdt, kind="ExternalInput").ap()
    wqT = nc.dram_tensor("wqT", [D, M], mdt, kind="ExternalInput").ap()
    wkT = nc.dram_tensor("wkT", [D, M], mdt, kind="ExternalInput").ap()
    wvT = nc.dram_tensor("wvT", [D, M], mdt, kind="ExternalInput").ap()
    bq = nc.dram_tensor("bq", [M], f32, kind="ExternalInput").ap()
    bk = nc.dram_tensor("bk", [M], f32, kind="ExternalInput").ap()
    bv = nc.dram_tensor("bv", [M], mdt, kind="ExternalInput").ap()
    woT = nc.dram_tensor("woT", [M, D], mdt, kind="ExternalInput").ap()
    ones32_dr = nc.dram_tensor("ones32", [1, 64], f32r, kind="ExternalInput").ap()
    out = nc.dram_tensor("out", [S, D], f32, kind="ExternalOutput").ap()

    with tile.TileContext(nc) as tc, ExitStack() as ctx:
        const_pool = ctx.enter_context(tc.tile_pool(name="const", bufs=1))
        x_pool = ctx.enter_context(tc.tile_pool(name="x", bufs=32))
        xv_pool = ctx.enter_context(tc.tile_pool(name="xv", bufs=16))
        e_pool = ctx.enter_context(tc.tile_pool(name="e", bufs=8))
        o_pool = ctx.enter_context(tc.tile_pool(name="o", bufs=2))
        r_pool = ctx.enter_context(tc.tile_pool(name="r", bufs=2))
        ps_a = ctx.enter_context(tc.tile_pool(name="ps_a", bufs=2, space="PSUM"))
        ps_b = ctx.enter_context(tc.tile_pool(name="ps_b", bufs=4, space="PSUM"))

        # ---- persistent SBUF tensors ----
        # weight slices as matmul lhsT, K-chunked: [p, kc, m]
        wq_sb = const_pool.tile([128, 8, M], mdt)
        wk_sb = const_pool.tile([128, 8, M], mdt)
        wv_sb = const_pool.tile([128, 8, M], mdt)
        for w_sb, w_dr in ((wq_sb, wqT), (wk_sb, wkT), (wv_sb, wvT)):
            nc.sync.dma_start(
                out=w_sb[:, :, :], in_=w_dr.rearrange("(kc p) m -> p kc m", p=128)
            )
        # out_proj rhs: [p, kc, j]
        wo_sb = const_pool.tile([128, 2, D], mdt)
        nc.sync.dma_start(
            out=wo_sb[:, :, :], in_=woT.rearrange("(kc p) j -> p kc j", p=128)
        )
        # per-partition biases for qpT/kpT: [p, mt]
        bq_sb = const_pool.tile([128, 2], f32)
        bk_sb = const_pool.tile([128, 2], f32)
        nc.sync.dma_start(out=bq_sb[:, :], in_=bq.rearrange("(mt p) -> p mt", p=128))
        nc.sync.dma_start(out=bk_sb[:, :], in_=bk.rearrange("(mt p) -> p mt", p=128))
        # bv as a K=1 matmul rhs row
        bv_sb = const_pool.tile([1, M], mdt)
        nc.sync.dma_start(out=bv_sb[:, :], in_=bv.rearrange("(a m) -> a m", a=1))
        ones_sb = const_pool.tile([1, 128], mdt)
        nc.vector.memset(ones_sb[:, :], 1.0)
        ones32_sb = const_pool.tile([1, 64], f32r)
        nc.sync.dma_start(out=ones32_sb[:, :], in_=ones32_dr[:, :])

        qpT = const_pool.tile([128, 2, S], mdt)   # [p, mt, s]
        kpT = const_pool.tile([128, 2, S], mdt)
        vp = const_pool.tile([128, KT, HC * 65], mdt)  # aug: 65-wide per head
        attnT = const_pool.tile([128, 2, S], mdt)

        # ones columns of the augmented V (once; head h at column h*65+64)
        nc.vector.memset(
            vp[:, :, :].rearrange("p kt (h c) -> p kt h c", c=65)[:, :, :, 64:65], 1.0
        )

        # ---- projections ----
        # x^T K-chunks stay resident (x_pool holds all 16 per tensor), so
        # each weight m-tile can be projected independently of load order.
        def load_half(x_dr, half, pool=None):
            fs = half * 1024
            chunks = []
            for kc in range(8):
                xt = (pool or x_pool).tile([128, 1024], mdt, tag="xchunk")
                nc.sync.dma_start(
                    out=xt[:, :], in_=x_dr[kc * 128:(kc + 1) * 128, fs:fs + 1024]
                )
                chunks.append((xt, fs))
            return chunks

        def load_chunks(x_dr, pool=None):
            return load_half(x_dr, 0, pool) + load_half(x_dr, 1, pool)

        def proj_half(chunks, w_sb, b_sb, p_sb, mt, half):
            fs = half * 1024
            for nch in range(2):
                ns = nch * 512
                ps = ps_b.tile([128, 512], f32, tag="ps_small", name="ps_p")
                for kc in range(8):
                    nc.tensor.matmul(
                        ps[:, :],
                        w_sb[:, kc, mt * 128:(mt + 1) * 128],
                        chunks[half * 8 + kc][0][:, ns:ns + 512],
                        start=(kc == 0),
                        stop=(kc == 7),
                    )
                nc.vector.tensor_scalar_add(
                    out=p_sb[:, mt, fs + ns:fs + ns + 512],
                    in0=ps[:, :],
                    scalar1=b_sb[:, mt:mt + 1],
                )

        def vp_group(chunks, kt):
            half, st = divmod(kt, 8)
            ps = ps_b.tile([128, 256], f32, tag="ps_small", name="ps_v")
            for kc in range(8):
                nc.tensor.matmul(
                    ps[:, 0:M],
                    chunks[half * 8 + kc][0][:, st * 128:(st + 1) * 128],
                    wv_sb[:, kc, :],
                    start=(kc == 0),
                    stop=False,
                )
            # bias via K=1 ones-row matmul
            nc.tensor.matmul(
                ps[:, 0:M],
                ones_sb[0:1, 0:128],
                bv_sb[0:1, :],
                start=False,
                stop=True,
            )
            nc.vector.tensor_copy(
                out=vp[:, kt, :].rearrange("p (h c) -> p h c", c=65)[:, :, 0:64],
                in_=ps[:, 0:M].rearrange("p (h c) -> p h c", c=64),
            )

        # interleave loads so scores for the first keys can start after
        # just the first half of xk + xq has landed
        chunks_k = load_half(xkT, 0)
        proj_half(chunks_k, wk_sb, bk_sb, kpT, 0, 0)
        chunks_q = load_half(xqT, 0)
        proj_half(chunks_q, wq_sb, bq_sb, qpT, 0, 0)
        chunks_k += load_half(xkT, 1)
        proj_half(chunks_k, wk_sb, bk_sb, kpT, 0, 1)
        chunks_q += load_half(xqT, 1)
        proj_half(chunks_q, wq_sb, bq_sb, qpT, 0, 1)
        for half in range(2):
            proj_half(chunks_k, wk_sb, bk_sb, kpT, 1, half)
            proj_half(chunks_q, wq_sb, bq_sb, qpT, 1, half)
        chunks_v = load_chunks(xvT, pool=xv_pool)

        # ---- attention + out-projection ----
        # The per-engine runtime schedule is static and in-order, so a
        # segment's normalization/out-projection is emitted INSIDE the next
        # segment's kt loop — its DVE-latency chain then overlaps the next
        # segment's compute instead of head-of-line blocking the PE queue.
        def flush_head(pair, qq, u, hh):
            qs = qq * 512
            rs = r_pool.tile([1, 512], f32r, tag="rs")
            with nc.allow_low_precision(reason="softmax denom"):
                nc.vector.tensor_copy(out=rs[:, :], in_=u[64:65, :])
            us = r_pool.tile([64, 512], f32, tag="us")
            nc.vector.tensor_copy(out=us[:, :], in_=u[0:64, :])
            rb = ps_b.tile([64, 512], f32, tag="ps_small")
            nc.tensor.matmul(
                rb[0:64, :], ones32_sb[0:1, 0:64], rs[0:1, :], start=True, stop=True
            )
            rbs = r_pool.tile([64, 512], f32, tag="rbs")
            nc.vector.reciprocal_approx_fast(out=rbs[:, :], in_=rb[0:64, :])
            with nc.allow_low_precision(reason="softmax normalize"):
                nc.vector.tensor_tensor(
                    out=attnT[hh * 64:hh * 64 + 64, pair, qs:qs + 512],
                    in0=us[0:64, :],
                    in1=rbs[0:64, :],
                    op=mybir.AluOpType.mult,
                )

        def outproj_stile(sg):
            ot = o_pool.tile([128, D], f32)
            for nch in range(2):
                ns = nch * 512
                po = ps_b.tile([128, 512], f32, tag="ps_small")
                for kc in range(2):
                    nc.tensor.matmul(
                        po[:, :],
                        attnT[:, kc, sg * 128:(sg + 1) * 128],
                        wo_sb[:, kc, ns:ns + 512],
                        start=(kc == 0),
                        stop=(kc == 1),
                    )
                nc.vector.tensor_copy(out=ot[:, ns:ns + 512], in_=po[:, :])
            nc.sync.dma_start(out=out[sg * 128:(sg + 1) * 128, :], in_=ot[:, :])

        pending_flush = None   # (pair, qq, u_tiles) awaiting normalization
        pending_out = []       # out-projection s-tiles ready to interleave
        for pair in range(2):
            for qq in range(QQ):
                qs = qq * 512
                u_tiles = []
                for h in (2 * pair, 2 * pair + 1):
                    u_tiles.append(
                        ps_b.tile([65, 512], f32, tag="ps_small", name=f"u_{qq}_{h}")
                    )
                for kt in range(KT):
                    ks = kt * 128
                    with tc.high_priority() if pair == 0 else _null_ctx():
                        sc = ps_a.tile([128, 2, 512], f32, tag="ps_main")
                        for hh in range(2):
                            po = hh * 64
                            nc.tensor.matmul(
                                sc[:, hh, :],
                                kpT[po:po + 64, pair, ks:ks + 128],
                                qpT[po:po + 64, pair, qs:qs + 512],
                                start=True,
                                stop=True,
                            )
                        et = e_pool.tile([128, 2, 512], mdt)
                        nc.scalar.activation(out=et[:, :, :], in_=sc[:, :, :], func=EXP)
                    if pair == 0 and qq == 0:
                        # V projection emitted just-in-time for its first consumer
                        vp_group(chunks_v, kt)
                    for hh in range(2):
                        h = 2 * pair + hh
                        nc.tensor.matmul(
                            u_tiles[hh][0:65, :],
                            vp[:, kt, h * 65:(h + 1) * 65],
                            et[:, hh, :],
                            start=(kt == 0),
                            stop=(kt == KT - 1),
                        )
                    # interleave the previous segment's epilogue
                    if pending_flush is not None and kt in (2, 4):
                        p_pair, p_qq, p_u = pending_flush
                        flush_head(p_pair, p_qq, p_u[kt // 2 - 1], kt // 2 - 1)
                        if kt == 4:
                            if p_pair == 1:
                                pending_out.extend(range(p_qq * 4, p_qq * 4 + 4))
                            pending_flush = None
                    elif pending_out and kt in (6, 9, 12, 15):
                        outproj_stile(pending_out.pop(0))
                pending_flush = (pair, qq, u_tiles)
        # tail: last segment's normalization + remaining out-projection
        p_pair, p_qq, p_u = pending_flush
        flush_head(p_pair, p_qq, p_u[0], 0)
        flush_head(p_pair, p_qq, p_u[1], 1)
        pending_out.extend(range(p_qq * 4, p_qq * 4 + 4))
        for sg in pending_out:
            outproj_stile(sg)

    nc.compile()
    return nc


def _get_compiled():
    global _compiled
    if _compiled is None:
        _compiled = _build_program()
    return _compiled


def _make_in_maps(q, k, v, in_proj_w, in_proj_b, out_proj_w):
    import ml_dtypes

    mdt_np = np.dtype(ml_dtypes.bfloat16) if MM_DT == "bfloat16" else np.float32

    def cvt(a):
        return np.ascontiguousarray(a).astype(mdt_np)

    xT = {}
    for b in range(B):
        xT[b] = (
            cvt(q[:, b, :].T),
            cvt(k[:, b, :].T),
            cvt(v[:, b, :].T),
        )
    scale = 1.0 / math.sqrt(DK)
    in_maps = []
    for c in range(N_CORES):
        b, g = divmod(c, HC)
        cols = slice(g * M, (g + 1) * M)
        in_maps.append({
            "xqT": xT[b][0],
            "xkT": xT[b][1],
            "xvT": xT[b][2],
            "wqT": cvt((in_proj_w[0 * D:1 * D][cols] * scale).T),
            "wkT": cvt(in_proj_w[1 * D:2 * D][cols].T),
            "wvT": cvt(in_proj_w[2 * D:3 * D][cols].T),
            "bq": np.ascontiguousarray(in_proj_b[0 * D:1 * D][cols] * scale),
            "bk": np.ascontiguousarray(in_proj_b[1 * D:2 * D][cols]),
            "bv": cvt(in_proj_b[2 * D:3 * D][cols]),
            "woT": cvt(out_proj_w[:, g * M:(g + 1) * M].T),
            "ones32": np.ones((1, 64), dtype=np.float32),
        })
    return in_maps


def kernel(q, k, v, in_proj_w, in_proj_b, out_proj_w, out_proj_b):
    from concourse.bass_utils import run_bass_kernel_spmd

    q = np.asarray(q, dtype=np.float32)
    k = np.asarray(k, dtype=np.float32)
    v = np.asarray(v, dtype=np.float32)
    in_proj_w = np.asarray(in_proj_w, dtype=np.float32)
    in_proj_b = np.asarray(in_proj_b, dtype=np.float32)
    out_proj_w = np.asarray(out_proj_w, dtype=np.float32)
    out_proj_b = np.asarray(out_proj_b, dtype=np.float32)

    nc = _get_compiled()
    in_maps = _make_in_maps(q, k, v, in_proj_w, in_proj_b, out_proj_w)

    res = run_bass_kernel_spmd(nc, in_maps, core_ids=list(range(N_CORES)))

    out = np.broadcast_to(out_proj_b.astype(np.float32), (S, B, D)).copy()
    for c in range(N_CORES):
        out[:, c // HC, :] += res.results[c]["out"]
    return out
